# revision 7
# baseline (speedup 1.0000x reference)
"""Grouped-experts MoE MLP (Aria) on 8 TRN2 NeuronCores.

Expert parallelism: 8 experts / 8 cores -> each core owns one expert's
weights (w1 [2048, 8192], w2 [4096, 2048]) and processes that expert's
token block (tokens are pre-sorted by expert, so routing is host-side
slicing). No device collectives needed.

This environment (axon-virtualized NeuronCores) executes long unrolled
instruction streams at ~1.8us/instruction past a few hundred matmuls,
so the kernel is one fused, software-pipelined For_i hardware loop
(4 iterations x 2 ping-pong groups; ~210 PE instructions per body,
IRAM-friendly with staggered_reset + PE branch hints).

Work is split into 8 "groups", one per 512-wide fc1 column pair
(proj tile np, gate tile np+8) + the matching 4 w2 k-tiles:

  group g body (bf16 matmuls, fp32 psum accumulate):
    DMA slot[g%2] <- [w1 proj g | w1 gate g | w2 j=4g..4g+3]  (6 MB)
    phase2 for group g-1: po[n] += hsegT[u].T @ other_slot.w2seg
        (other slot still holds group g-1's weights - no DMA wait)
    fc1 proj/gate psum [tok, 512] = sum_k xt[k].T @ slot.w1seg[k]
    hidden = silu(proj) * gate        (ACT + DVE, bf16)
    hsegT = PE-transpose(hidden)      (4 transposes via identity)
  epilogue: phase2 for group 7 from the resident slot.

The po accumulators live in PSUM across the whole loop, seeded by
start=True matmuls on zeros (start flags must be static in the loop).

Host pre-arranges weights partition-major so each group's DMA is one
contiguous 48KB-per-partition block, and casts to bf16 (halves the
HBM traffic; the memory-bound roofline is weight streaming).
"""

import sys
import types

sys.path.insert(0, "/opt/trn_rl_repo")

# This axon deployment ships without antenv.axon_hooks; shim it so
# bass_utils' trace path degrades gracefully instead of ImportError-ing.
try:
    import antenv  # noqa: F401

    if "antenv.axon_hooks" not in sys.modules:
        _hooks = types.ModuleType("antenv.axon_hooks")
        _hooks.get_axon_ntff_profile_hook = lambda: None
        sys.modules["antenv.axon_hooks"] = _hooks
except ImportError:
    pass

from contextlib import ExitStack

import ml_dtypes
import numpy as np

import concourse.bass as bass  # noqa: F401
import concourse.tile as tile
from concourse import bacc, mybir
from concourse.bass import ds
from concourse.bass_utils import run_bass_kernel_spmd
from concourse.masks import make_identity

NUM_TOKENS = 1024
HIDDEN = 2048
INTER = 4096
EXPERTS = 8
N_CORES = 8
P = 128
T = 128  # tokens per core (padded)
KT1 = HIDDEN // P  # 16 k-tiles for matmul 1
NT1 = (2 * INTER) // 512  # 16 fc1 column tiles of 512
NG = NT1 // 2  # 8 proj/gate pair groups
JT = INTER // P  # 32 inter k-tiles for matmul 2
NT2 = HIDDEN // 512  # 4 output column tiles of 512
GCOL = KT1 * 512  # 8192 cols per w1 segment
W2COL = 4 * HIDDEN  # 8192 cols per w2 segment
GTOT = 2 * GCOL + W2COL  # 24576 cols per combined group

BF16 = mybir.dt.bfloat16
F32 = mybir.dt.float32

_CACHE = {}


def _emit_group(nc, xt, ident, slot, other, po, hsegT, psum1, trp, spool, hpool,
                first_p2: bool):
    """Emit one group's work. phase2 for the PREVIOUS group reads
    `other`'s w2 segment and the current hsegT contents; then fc1 for
    this group from `slot`, silu*gate, and transposes into hsegT."""
    if not first_p2:
        for u in range(4):
            for n in range(NT2):
                nc.tensor.matmul(
                    po[n][:],
                    lhsT=hsegT[:, u * T : (u + 1) * T],
                    rhs=other[:, 2 * GCOL + u * HIDDEN + n * 512 :
                              2 * GCOL + u * HIDDEN + (n + 1) * 512],
                    start=False,
                    stop=False,
                    skip_group_check=True,
                )

    pa = psum1.tile([T, 512], F32, tag="ps1t")
    pb = psum1.tile([T, 512], F32, tag="ps1t")
    for k in range(KT1):
        nc.tensor.matmul(
            pa[:],
            lhsT=xt[:, k * T : (k + 1) * T],
            rhs=slot[:, k * 512 : (k + 1) * 512],
            start=(k == 0),
            stop=(k == KT1 - 1),
        )
    for k in range(KT1):
        nc.tensor.matmul(
            pb[:],
            lhsT=xt[:, k * T : (k + 1) * T],
            rhs=slot[:, GCOL + k * 512 : GCOL + (k + 1) * 512],
            start=(k == 0),
            stop=(k == KT1 - 1),
        )
    sa = spool.tile([T, 512], F32, tag="silu")
    nc.scalar.activation(sa[:], pa[:], mybir.ActivationFunctionType.Silu)
    hseg = hpool.tile([T, 512], BF16, tag="hseg")
    nc.vector.tensor_mul(hseg[:], sa[:], pb[:])

    for half in range(2):
        tp = trp.tile([P, 2 * P], BF16, tag="trt")
        for s in range(2):
            u = 2 * half + s
            nc.tensor.transpose(
                tp[:, s * P : (s + 1) * P],
                hseg[:, u * P : (u + 1) * P],
                ident[:],
            )
        nc.vector.tensor_copy(
            hsegT[:, half * 2 * T : (half + 1) * 2 * T], tp[:]
        )


def _build(reps: int = 1):
    nc = bacc.Bacc(
        "TRN2", target_bir_lowering=False, debug=False, num_devices=N_CORES
    )
    xt_d = nc.dram_tensor("xt", [P, KT1 * T], BF16, kind="ExternalInput").ap()
    wc_d = nc.dram_tensor("wc", [P, NG * GTOT], BF16, kind="ExternalInput").ap()
    out_d = nc.dram_tensor("out", [T, HIDDEN], F32, kind="ExternalOutput").ap()

    with tile.TileContext(nc) as tc:
        with ExitStack() as ctx:
            xpool = ctx.enter_context(tc.tile_pool(name="x", bufs=1))
            ipool = ctx.enter_context(tc.tile_pool(name="id", bufs=1))
            wpool = ctx.enter_context(tc.tile_pool(name="wc", bufs=1))
            spool = ctx.enter_context(tc.tile_pool(name="s", bufs=2))
            hpool = ctx.enter_context(tc.tile_pool(name="h", bufs=2))
            htp = ctx.enter_context(tc.tile_pool(name="ht", bufs=1))
            opool = ctx.enter_context(tc.tile_pool(name="o", bufs=1))
            psum1 = ctx.enter_context(tc.tile_pool(name="ps1", bufs=2, space="PSUM"))
            trp = ctx.enter_context(tc.tile_pool(name="tr", bufs=2, space="PSUM"))
            psum2 = ctx.enter_context(tc.tile_pool(name="ps2", bufs=1, space="PSUM"))

            xt = xpool.tile([P, KT1 * T], BF16)
            nc.sync.dma_start(xt[:], xt_d[:, :])
            ident = ipool.tile([P, P], BF16)
            make_identity(nc, ident[:])
            zt = ipool.tile([P, 512], BF16)
            nc.vector.memset(zt[:], 0.0)

            for _rep in range(reps):
                slotA = wpool.tile([P, GTOT], BF16, tag="slotA")
                slotB = wpool.tile([P, GTOT], BF16, tag="slotB")
                hsegT = htp.tile([P, 4 * T], BF16, tag="hsegT")
                # first-body reads of slotB.w2seg/hsegT multiply by zeros,
                # but uninitialized SBUF could hold NaN bit patterns.
                nc.vector.memset(hsegT[:], 0.0)
                nc.vector.memset(slotB[:, 2 * GCOL :], 0.0)

                po = [psum2.tile([P, 512], F32, name=f"po{n}") for n in range(NT2)]
                for n in range(NT2):
                    nc.tensor.matmul(
                        po[n][:], lhsT=zt[:, :P], rhs=zt[:, :512],
                        start=True, stop=False, skip_group_check=True,
                    )

                with tc.For_i(
                    0, NG // 2, 1,
                    staggered_reset=True,
                    hint_engines=(mybir.EngineType.PE,),
                ) as m:
                    nc.sync.dma_start(
                        slotA[:], wc_d[:, ds((m * 2) * GTOT, GTOT)]
                    )
                    _emit_group(nc, xt, ident, slotA, slotB, po, hsegT,
                                psum1, trp, spool, hpool, first_p2=False)
                    nc.sync.dma_start(
                        slotB[:], wc_d[:, ds((m * 2 + 1) * GTOT, GTOT)]
                    )
                    _emit_group(nc, xt, ident, slotB, slotA, po, hsegT,
                                psum1, trp, spool, hpool, first_p2=False)

                # epilogue: phase2 for group 7 (weights resident in slotB)
                for u in range(4):
                    for n in range(NT2):
                        nc.tensor.matmul(
                            po[n][:],
                            lhsT=hsegT[:, u * T : (u + 1) * T],
                            rhs=slotB[:, 2 * GCOL + u * HIDDEN + n * 512 :
                                      2 * GCOL + u * HIDDEN + (n + 1) * 512],
                            start=False,
                            stop=False,
                            skip_group_check=True,
                        )
                for n in range(NT2):
                    nc.tensor.matmul(
                        po[n][:], lhsT=zt[:, :P], rhs=zt[:, :512],
                        start=False, stop=True, skip_group_check=True,
                    )

                osb = opool.tile([T, HIDDEN], F32, tag="osb")
                for n in range(NT2):
                    nc.scalar.copy(osb[:, n * 512 : (n + 1) * 512], po[n][:])
                nc.sync.dma_start(out_d[:, :], osb[:])

    nc.compile()
    return nc


def _get_nc(reps: int = 1):
    key = ("nc", reps)
    if key not in _CACHE:
        _CACHE[key] = _build(reps)
    return _CACHE[key]


def _prep_token_block(x_block: np.ndarray) -> np.ndarray:
    """[T, HIDDEN] f32 -> xt layout [P, KT1*T] bf16 where
    xt[p, k*T + t] = x_block[t, k*P + p]."""
    a = np.ascontiguousarray(
        x_block.T.reshape(KT1, P, T).transpose(1, 0, 2).reshape(P, KT1 * T)
    )
    return a.astype(ml_dtypes.bfloat16)


def _prep_wc(w1_e: np.ndarray, w2_e: np.ndarray) -> np.ndarray:
    """w1 [HIDDEN, 2*INTER], w2 [INTER, HIDDEN] f32 -> combined
    [P, NG*GTOT] bf16. Group g = [w1 proj g | w1 gate g | w2 j=4g..4g+3],
    w1 segs laid out (k, c) -> k*512+c, w2 seg (u, c) -> u*HIDDEN+c."""
    a1 = w1_e.reshape(KT1, P, NT1, 512).transpose(1, 2, 0, 3)  # [p, n, k, c]
    a1 = a1.reshape(P, NT1, GCOL)
    a2 = w2_e.reshape(NG, 4, P, HIDDEN).transpose(2, 0, 1, 3)  # [p, g, u, c]
    a2 = a2.reshape(P, NG, W2COL)
    groups = [
        np.concatenate([a1[:, g], a1[:, g + NG], a2[:, g]], axis=1)
        for g in range(NG)
    ]
    return np.ascontiguousarray(np.concatenate(groups, axis=1)).astype(
        ml_dtypes.bfloat16
    )


def _run_device(in_maps):
    nc = _get_nc()
    res = run_bass_kernel_spmd(nc, in_maps, core_ids=list(range(N_CORES)))
    return [r["out"] for r in res.results]


def kernel(permuted_tokens, w1, w2, tokens_per_expert):
    permuted_tokens = np.asarray(permuted_tokens, dtype=np.float32)
    w1 = np.asarray(w1, dtype=np.float32)
    w2 = np.asarray(w2, dtype=np.float32)
    counts = np.asarray(tokens_per_expert).astype(np.int64)

    n = permuted_tokens.shape[0]
    bounds = np.minimum(np.cumsum(counts), n)
    starts = np.concatenate([[0], bounds[:-1]])
    eff_counts = np.maximum(bounds - starts, 0)

    wc_maps = [_prep_wc(w1[e], w2[e]) for e in range(EXPERTS)]

    out = np.zeros((n, HIDDEN), dtype=np.float32)
    rounds = int(max(1, -(-int(eff_counts.max()) // T)))
    for r in range(rounds):
        in_maps = []
        chunk_info = []
        for e in range(EXPERTS):
            c0 = starts[e] + r * T
            cnt = int(min(max(eff_counts[e] - r * T, 0), T))
            blk = np.zeros((T, HIDDEN), dtype=np.float32)
            if cnt > 0:
                blk[:cnt] = permuted_tokens[c0 : c0 + cnt]
            chunk_info.append((c0, cnt))
            in_maps.append({"xt": _prep_token_block(blk), "wc": wc_maps[e]})
        outs = _run_device(in_maps)
        for e in range(EXPERTS):
            c0, cnt = chunk_info[e]
            if cnt > 0:
                out[c0 : c0 + cnt] = outs[e][:cnt]
    return out
